# revision 27
# baseline (speedup 1.0000x reference)
"""Trainium2 Bass kernel for nn_Net_SDE: 48-step neural SDE Monte-Carlo pricer.

Data-parallel over the 131072 MC samples across 8 NeuronCores (16384/core).

Per core the 48 SDE steps are fully unrolled (no Tile loop back-edge barriers)
and each step is processed in four 4096-sample "quarters" so that the
inter-step boundary work (state update, layout shuffles) of quarter q hides
under the MLP streams of quarter q+1.

Within a quarter, the 4 MLPs are evaluated layer-wise as weight-stationary
streams of 8 N=512 fp16 matmuls into two [128, 2048] (4-bank) PSUM tiles;
each PSUM tile is drained by a single FD=2048 bias+relu op that alternates
between the vector (DVE) and scalar (ACT) engines — the two drain engines
together are the throughput limit of the whole kernel.  The first layer is
row-tiled (tile_position=(32n,0), K=2) and the output layer is col-tiled
(tile_position=(0,32n), M=1) so the four nets share the PE array.

Payoffs use put-call parity: only call partials relu(S-K) and sum(S) are
computed on-device (21 ops per maturity instead of 40); the host derives the
put columns exactly as relu(K-S) = relu(S-K) - (S-K).
"""
import numpy as np
from contextlib import ExitStack

import orjson

import concourse.bass as bass
import concourse.tile as tile
from concourse import mybir

F16 = mybir.dt.float16
F32 = mybir.dt.float32
AF = mybir.ActivationFunctionType
OP = mybir.AluOpType

MC = 131072
N_STEPS = 48
N_CORES = 8
MCL = MC // N_CORES          # 16384 samples per core
GF = 128                     # state grid: [128 partitions, GF free]
NQ = 4                       # quarters per step
QF = GF // NQ                # grid cols per quarter (32 -> 4096 samples)
QS = 128 * QF                # samples per quarter
CH = 512                     # matmul moving-dim chunk
PMW = 1024                   # psum tile free width (2 banks)
NSEG = QS // PMW             # psum tiles per (net, layer) stream (4)
CPS = PMW // CH              # matmul chunks per psum tile (2)

STRIKES_CALL = np.array([100., 105., 110., 115., 120., 125., 130., 135., 140., 145.], np.float32)
STRIKES_PUT = np.array([55., 60., 65., 70., 75., 80., 85., 90., 95., 100.], np.float32)


# ---------------------------------------------------------------------------
# Workaround: this walrus build accepts only ONE sync-wait command per
# instruction. Split any instruction with more waits into preceding
# same-engine Drain (ctrl no-op) instructions, one wait each — same-engine
# FIFO order makes this semantically identical.
def _split_sync_waits(bir_json: bytes) -> bytes:
    bir = orjson.loads(bir_json)
    for fn in bir.get("functions", []):
        for bb in fn.get("blocks", []):
            out = []
            changed = False
            for ins in bb.get("instructions", []):
                si = ins.get("sync_info") or {}
                waits = si.get("on_wait") or []
                if len(waits) > 1:
                    changed = True
                    for ci, w in enumerate(waits[:-1]):
                        out.append({
                            "name": f"{ins['name']}_sw{ci}",
                            "opcode": "Drain",
                            "engine": ins.get("engine", "SP"),
                            "ins": [], "outs": [],
                            "debug": ins.get("debug"),
                            "sync_info": {"on_update": [], "on_wait": [w]},
                        })
                    si["on_wait"] = waits[-1:]
                    ins["sync_info"] = si
                out.append(ins)
            if changed:
                bb["instructions"] = out
    return orjson.dumps(bir)


def _install_sync_split():
    import concourse.bass_utils as bu
    import concourse.bass2jax as b2j
    if getattr(bu, "_sync_split_installed", False):
        return
    orig = bu.compile_bir_kernel

    def patched(bir_json, tmpdir, neff_name="file.neff"):
        return orig(_split_sync_waits(bir_json), tmpdir, neff_name=neff_name)

    bu.compile_bir_kernel = patched
    bu._sync_split_installed = True
    if getattr(b2j, "compile_bir_kernel", None) is orig:
        b2j.compile_bir_kernel = patched


def build_nc(idx_steps, c0, bo0, bo1h, bo2, bo3, cS, cV, n_steps=N_STEPS,
             repeat=1):
    """Build the single-core Bass program (SPMD: all cores run the same code).

    idx_steps: list of 24 ints (step whose post-update S feeds output row i).
    c0 = 1 + r*h. bo* = output biases (net 1 pre-scaled by h).
    """
    nc = bass.Bass()

    z_in = nc.declare_dram_parameter("z", [n_steps, MCL], F32, isOutput=False)
    z1_in = nc.declare_dram_parameter("z1", [n_steps, MCL], F32, isOutput=False)
    wiT_in = nc.declare_dram_parameter("wiT", [128, 256], F16, isOutput=False)
    whT_in = nc.declare_dram_parameter("whT", [128, 1536], F16, isOutput=False)
    woT_in = nc.declare_dram_parameter("woT", [128, 16], F16, isOutput=False)
    b1_in = nc.declare_dram_parameter("b1", [128, 4 * n_steps], F32, isOutput=False)
    bh_in = nc.declare_dram_parameter("bh", [128, 12], F32, isOutput=False)
    strk_in = nc.declare_dram_parameter("strk", [128, 20], F32, isOutput=False)
    n_mat = len(idx_steps)
    acc_out = nc.declare_dram_parameter("acc", [128, 21 * n_mat], F32, isOutput=True)

    # maturity -> list of output rows (handles duplicate indices)
    mat_map = {}
    for i, st in enumerate(idx_steps):
        mat_map.setdefault(st, []).append(i)

    # SWDGE (Pool-issued) DMAs fail walrus codegen inside a For_i loop
    # ("ISA wrong length"), so the repeat timing build falls back to SP.
    ldma = None  # set below once nc engines exist

    with tile.TileContext(nc) as tc, ExitStack() as ctx:
        ldma = nc.gpsimd.dma_start if repeat == 1 else nc.sync.dma_start
        consts = ctx.enter_context(tc.tile_pool(name="consts", bufs=1))
        persist = ctx.enter_context(tc.tile_pool(name="persist", bufs=1))
        hpool = ctx.enter_context(tc.tile_pool(name="hpool", bufs=2))
        inppool = ctx.enter_context(tc.tile_pool(name="inppool", bufs=5))
        orowpool = ctx.enter_context(tc.tile_pool(name="orowpool", bufs=3))
        outspool = ctx.enter_context(tc.tile_pool(name="outspool", bufs=3))
        zpool = ctx.enter_context(tc.tile_pool(name="zpool", bufs=3))
        updpool = ctx.enter_context(tc.tile_pool(name="updpool", bufs=2))
        junkpool = ctx.enter_context(tc.tile_pool(name="junkpool", bufs=4))
        psmm = ctx.enter_context(tc.tile_pool(name="psmm", bufs=4, space="PSUM"))

        # ---- constants ----
        wiT = consts.tile([128, 256], F16)
        nc.sync.dma_start(out=wiT, in_=wiT_in[:, :])
        whT = consts.tile([128, 1536], F16)
        nc.sync.dma_start(out=whT, in_=whT_in[:, :])
        woT = consts.tile([128, 16], F16)
        nc.sync.dma_start(out=woT, in_=woT_in[:, :])
        b1 = consts.tile([128, 4 * n_steps], F32)
        nc.sync.dma_start(out=b1, in_=b1_in[:, :])
        bh = consts.tile([128, 12], F32)
        nc.sync.dma_start(out=bh, in_=bh_in[:, :])
        strk = consts.tile([128, 20], F32)
        nc.sync.dma_start(out=strk, in_=strk_in[:, :])

        # ---- persistent state ----
        S = persist.tile([128, GF], F32)
        V = persist.tile([128, GF], F32)
        S16 = persist.tile([128, GF], F16)
        V16 = persist.tile([128, GF], F16)
        acc = persist.tile([128, 21 * n_mat], F32)

        nc.vector.memset(S[:, :], cS)
        nc.vector.memset(V[:, :], cV)
        nc.vector.memset(S16[:, :], 0.0)
        nc.vector.memset(V16[:, :], 0.0)

        ua = updpool.tile([128, QF], F32, tag="ua")
        ub = updpool.tile([128, QF], F32, tag="ub")
        uc = updpool.tile([128, QF], F32, tag="uc")
        ud = updpool.tile([128, QF], F32, tag="ud")

        # drain engine alternation: DVE is slightly slower per drain, so it
        # takes 15 of every 32 big drains; payoff/small ops alternate evenly.
        state = {"k": 0, "pj": 0}

        def drain_relu(dst, pm_ap, bias_ap):
            k = state["k"]
            state["k"] += 1
            if (k * 15) // 32 != ((k + 1) * 15) // 32:
                nc.vector.tensor_scalar(out=dst, in0=pm_ap, scalar1=bias_ap,
                                        scalar2=0.0, op0=OP.add, op1=OP.max)
            else:
                nc.scalar.activation(dst, pm_ap, AF.Relu, bias=bias_ap, scale=1.0)

        def drain_copy(dst, pm_ap):
            k = state["k"]
            state["k"] += 1
            if (k * 15) // 32 != ((k + 1) * 15) // 32:
                nc.vector.tensor_copy(dst, pm_ap)
            else:
                nc.scalar.copy(dst, pm_ap)

        # initial first-layer input: centered state is exactly zero
        inp_tiles = []
        for q in range(NQ):
            t0 = inppool.tile([128, QS], F16, tag="inp")
            nc.vector.memset(t0[:, :], 0.0)
            inp_tiles.append(t0)

        def emit_payoff(row):
            # ACT only: DVE's accum_out does not produce free-dim sums on
            # this stack (verified empirically — it returns the last element).
            base = 21 * row
            for j in range(20):
                junk = junkpool.tile([128, GF], F32, tag="junk")
                col = acc[:, base + j:base + j + 1]
                nc.scalar.activation(junk, S, AF.Relu, bias=strk[:, j:j + 1],
                                     scale=1.0, accum_out=col)
            junk = junkpool.tile([128, GF], F32, tag="junk")
            nc.scalar.activation(junk, S, AF.Copy,
                                 accum_out=acc[:, base + 20:base + 21])

        # ---- main SDE loop (fully unrolled; repeat>1 is timing-only) ----
        rep_ctx = (tc.For_i(0, repeat, 1) if repeat > 1 else None)
        if rep_ctx is not None:
            rep_ctx.__enter__()
        for t in range(n_steps):
            z_t = zpool.tile([128, GF], F32, tag="z")
            ldma(out=z_t[:, :], in_=z_in[t:t + 1, :])
            z1_t = zpool.tile([128, GF], F32, tag="z1")
            ldma(out=z1_t[:, :], in_=z1_in[t:t + 1, :])

            for q in range(NQ):
                qs = slice(QF * q, QF * q + QF)
                inp_q = inp_tiles[q]
                # -- first layer: 2-way row-tiled K=2 (strips 0 and 32);
                # net pairs (0,1) then (2,3) run concurrently on the PE with
                # their two psum tiles draining on DVE and ACT in parallel --
                hcur = [None] * 4
                for na, nb in ((0, 1), (2, 3)):
                    ha = hpool.tile([128, QS], F16, tag=f"h{na}")
                    hb = hpool.tile([128, QS], F16, tag=f"h{nb}")
                    for seg in range(NSEG):
                        pma = psmm.tile([128, PMW], F32, tag="pm")
                        pmb = psmm.tile([128, PMW], F32, tag="pm")
                        for c in range(CPS):
                            col = seg * PMW + c * CH
                            for n, pm in ((na, pma), (nb, pmb)):
                                strip = 32 * (n % 2)
                                wcol = 128 * (n // 2)
                                nc.tensor.matmul(
                                    pm[:, c * CH:(c + 1) * CH],
                                    lhsT=wiT[strip:strip + 2, wcol:wcol + 128],
                                    rhs=inp_q[strip:strip + 2, col:col + CH],
                                    start=True, stop=True, tile_position=(strip, 0))
                        for n, pm, hn in ((na, pma, ha), (nb, pmb, hb)):
                            drain_relu(hn[:, seg * PMW:(seg + 1) * PMW], pm[:, :],
                                       b1[:, 4 * t + n:4 * t + n + 1])
                    hcur[na] = ha
                    hcur[nb] = hb
                # -- hidden layers --
                for l in range(3):
                    for n in range(4):
                        hn = hpool.tile([128, QS], F16, tag=f"h{n}")
                        w_sl = slice((n * 3 + l) * 128, (n * 3 + l + 1) * 128)
                        for seg in range(NSEG):
                            pm = psmm.tile([128, PMW], F32, tag="pm")
                            for c in range(CPS):
                                col = seg * PMW + c * CH
                                nc.tensor.matmul(
                                    pm[:, c * CH:(c + 1) * CH],
                                    lhsT=whT[:, w_sl],
                                    rhs=hcur[n][:, col:col + CH],
                                    start=True, stop=True)
                            drain_relu(hn[:, seg * PMW:(seg + 1) * PMW], pm[:, :],
                                       bh[:, n * 3 + l:n * 3 + l + 1])
                        hcur[n] = hn
                # -- output layer: M=4 zero-padded weights, PSUM-accumulated
                # across the 4 nets so outputs land on partitions 0-3 --
                orow = orowpool.tile([4, QS], F32, tag="orow")
                for seg in range(NSEG):
                    pmo = psmm.tile([128, PMW], F32, tag="pm")
                    for c in range(CPS):
                        for n in range(4):
                            col = seg * PMW + c * CH
                            nc.tensor.matmul(
                                pmo[0:4, c * CH:(c + 1) * CH],
                                lhsT=woT[:, 4 * n:4 * n + 4],
                                rhs=hcur[n][:, col:col + CH],
                                start=(n == 0), stop=(n == 3))
                    drain_copy(orow[0:4, seg * PMW:(seg + 1) * PMW],
                               pmo[0:4, :])
                # -- scatter net outputs back to the state grid --
                og = []
                for n in range(4):
                    g = outspool.tile([128, QF], F32, tag=f"og{n}")
                    ldma(out=g[:, :], in_=orow[n:n + 1, :])
                    og.append(g)
                # -- state update (fp32, DVE) --
                Ssl = S[:, qs]
                Vsl = V[:, qs]
                zsl = z_t[:, qs]
                z1sl = z1_t[:, qs]
                # state update: scalar_tensor_tensor must stay on DVE (walrus
                # rejects it on Pool); plain tensor ops go to the idle GPSIMD
                # S' = relu(c0*S + (diff+bo0)*dW)
                nc.vector.scalar_tensor_tensor(out=ua, in0=og[0], scalar=bo0,
                                               in1=zsl, op0=OP.add, op1=OP.mult)
                nc.vector.scalar_tensor_tensor(out=ub, in0=Ssl, scalar=c0,
                                               in1=ua, op0=OP.mult, op1=OP.add)
                nc.vector.tensor_scalar(out=Ssl, in0=ub, scalar1=0.0,
                                        scalar2=None, op0=OP.max)
                # V' = V + (driftV*h+bo1h) + (diffV+bo2)*dW + (diffV1+bo3)*dW1
                nc.vector.scalar_tensor_tensor(out=ua, in0=og[1], scalar=bo1h,
                                               in1=Vsl, op0=OP.add, op1=OP.add)
                nc.vector.scalar_tensor_tensor(out=ub, in0=og[2], scalar=bo2,
                                               in1=zsl, op0=OP.add, op1=OP.mult)
                nc.vector.scalar_tensor_tensor(out=uc, in0=og[3], scalar=bo3,
                                               in1=z1sl, op0=OP.add, op1=OP.mult)
                nc.vector.tensor_tensor(out=ud, in0=ua, in1=ub, op=OP.add)
                nc.vector.tensor_tensor(out=Vsl, in0=ud, in1=uc, op=OP.add)
                # centered fp16 copies for the next step's first layer
                nc.vector.tensor_scalar(out=S16[:, qs], in0=Ssl, scalar1=cS,
                                        scalar2=None, op0=OP.subtract)
                nc.vector.tensor_scalar(out=V16[:, qs], in0=Vsl, scalar1=cV,
                                        scalar2=None, op0=OP.subtract)
                # build next step's first-layer input rows (4 replicas for
                # the row-tiled first layer)
                if t + 1 < n_steps or repeat > 1:
                    ninp = inppool.tile([128, QS], F16, tag="inp")
                    for strip in (0, 32):
                        ldma(out=ninp[strip:strip + 1, :], in_=S16[:, qs])
                        ldma(out=ninp[strip + 1:strip + 2, :], in_=V16[:, qs])
                    inp_tiles[q] = ninp

            # payoffs for maturities at this step (post-update S)
            for row in mat_map.get(t, []):
                emit_payoff(row)
        if rep_ctx is not None:
            rep_ctx.__exit__(None, None, None)

        nc.sync.dma_start(out=acc_out[:, :], in_=acc)

    return nc


def _prep_inputs(S0, V0, rate, z, z1, indices, timegrid, Wi, bi, Wh, bh, Wo, bo,
                 n_steps=None):
    """Host-side preprocessing. Returns (build args, per-core inputs, disc)."""
    S0v = float(np.asarray(S0).reshape(-1)[0])
    V0v = float(np.asarray(V0).reshape(-1)[0])
    r = float(np.asarray(rate).reshape(-1)[0])
    z = np.asarray(z, np.float32)
    z1 = np.asarray(z1, np.float32)
    if n_steps is None:
        n_steps = z.shape[1]
    tg = np.asarray(timegrid, np.float64)
    h = float(tg[1] - tg[0])
    sqh = float(np.sqrt(h))
    c0 = 1.0 + r * h

    Wi = np.asarray(Wi, np.float32)
    bi = np.asarray(bi, np.float32)
    Wh = np.asarray(Wh, np.float32)
    bhv = np.asarray(bh, np.float32)
    Wo = np.asarray(Wo, np.float32).copy()
    bo = np.asarray(bo, np.float32).copy()
    # driftV net (index 1) is only ever used multiplied by h -> fold h into it
    Wo[1] *= h
    bo0, bo1h, bo2, bo3 = (float(bo[0, 0]), float(bo[1, 0]) * h,
                           float(bo[2, 0]), float(bo[3, 0]))

    cS, cV = S0v, V0v    # centering constants for fp16 inputs
    # first-layer bias with t-term and centering folded in: [4, n_steps, 128]
    t_vals = tg[:n_steps].astype(np.float32)
    b1 = (bi[:, None, :] + t_vals[None, :, None] * Wi[:, 0][:, None, :]
          + cS * Wi[:, 1][:, None, :] + cV * Wi[:, 2][:, None, :])
    # device layout: [128 features, n_steps*4] with col t*4+n
    b1_dev = np.ascontiguousarray(b1.transpose(2, 1, 0).reshape(128, n_steps * 4),
                                  np.float32)

    # first-layer weights, 2-way row strips: net n lives at partition rows
    # 32*(n%2).. and weight cols 128*(n//2)..
    wiT_dev = np.zeros((128, 256), np.float16)
    for n in range(4):
        strip, wcol = 32 * (n % 2), 128 * (n // 2)
        wiT_dev[strip:strip + 2, wcol:wcol + 128] = Wi[n, 1:3, :].astype(np.float16)
    whT_dev = np.ascontiguousarray(
        Wh.transpose(2, 0, 1, 3).reshape(128, 12 * 128), np.float16)
    # output weights, zero-padded to M=4 per net: cols 4n..4n+4, col 4n+j
    # is Wo[n] when j == n else 0 (outputs accumulate on psum partitions 0-3)
    woT_dev = np.zeros((128, 16), np.float16)
    for n in range(4):
        woT_dev[:, 4 * n + n] = Wo[n, :, 0].astype(np.float16)
    bh_dev = np.ascontiguousarray(bhv.transpose(2, 0, 1).reshape(128, 12), np.float32)

    strk_dev = np.ascontiguousarray(
        np.tile(np.concatenate([-STRIKES_CALL, -STRIKES_PUT])[None, :], (128, 1)),
        np.float32)

    idx = np.asarray(indices).astype(np.int64).reshape(-1)
    idx_steps = [int((v - 1) % n_steps) for v in idx]
    disc = np.exp(-r * 2.0 * idx.astype(np.float64) / n_steps)

    in_maps = []
    for k in range(N_CORES):
        sl = slice(k * MCL, (k + 1) * MCL)
        in_maps.append({
            "z": np.ascontiguousarray((z[sl, :n_steps] * sqh).T, np.float32),
            "z1": np.ascontiguousarray((z1[sl, :n_steps] * sqh).T, np.float32),
            "wiT": wiT_dev, "whT": whT_dev, "woT": woT_dev,
            "b1": b1_dev, "bh": bh_dev, "strk": strk_dev,
        })
    build_args = dict(idx_steps=idx_steps, c0=c0, bo0=bo0, bo1h=bo1h,
                      bo2=bo2, bo3=bo3, cS=cS, cV=cV, n_steps=n_steps)
    return build_args, in_maps, disc


def _combine(results, disc, idx_steps):
    """Sum per-core [128, 21*n_mat] partials into the [96, 10] output."""
    n_mat = len(idx_steps)
    total = np.zeros((128, 21 * n_mat), np.float64)
    for res in results:
        total += np.asarray(res["acc"], np.float64)
    cols = total.sum(axis=0).reshape(n_mat, 21)
    calls_c = cols[:, 0:10]                  # sum relu(S - Kc)
    calls_p = cols[:, 10:20]                 # sum relu(S - Kp)
    sumS = cols[:, 20:21]                    # sum S
    kc = STRIKES_CALL.astype(np.float64)[None, :]
    kp = STRIKES_PUT.astype(np.float64)[None, :]
    # relu(K - S) = relu(S - K) - S + K  (summed over MC samples)
    puts_c = calls_c - sumS + MC * kc
    puts_p = calls_p - sumS + MC * kp
    out = np.concatenate([calls_c, puts_p, calls_p, puts_c], axis=0) / MC
    out = out * np.concatenate([disc] * 4)[:, None]
    return out.astype(np.float32)


def kernel(**inputs) -> np.ndarray:
    from concourse.bass_utils import run_bass_kernel_spmd
    _install_sync_split()
    build_args, in_maps, disc = _prep_inputs(**inputs)
    nc = build_nc(**build_args)
    res = run_bass_kernel_spmd(nc, in_maps, list(range(N_CORES)))
    return _combine(res.results, disc, build_args["idx_steps"])


# revision 28
# speedup vs baseline: 3.8339x; 3.8339x over previous
"""Trainium2 Bass kernel for nn_Net_SDE: 48-step neural SDE Monte-Carlo pricer.

Data-parallel over the 131072 MC samples across 8 NeuronCores (16384/core).

Per core the 48 SDE steps are fully unrolled (no Tile loop back-edge barriers)
and each step is processed in four 4096-sample "quarters" so that the
inter-step boundary work (state update, layout shuffles) of quarter q hides
under the MLP streams of quarter q+1.

Within a quarter, the 4 MLPs are evaluated layer-wise as weight-stationary
streams of 8 N=512 fp16 matmuls into two [128, 2048] (4-bank) PSUM tiles;
each PSUM tile is drained by a single FD=2048 bias+relu op that alternates
between the vector (DVE) and scalar (ACT) engines — the two drain engines
together are the throughput limit of the whole kernel.  The first layer is
row-tiled (tile_position=(32n,0), K=2) and the output layer is col-tiled
(tile_position=(0,32n), M=1) so the four nets share the PE array.

Payoffs use put-call parity: only call partials relu(S-K) and sum(S) are
computed on-device (21 ops per maturity instead of 40); the host derives the
put columns exactly as relu(K-S) = relu(S-K) - (S-K).
"""
import numpy as np
from contextlib import ExitStack

import orjson

import concourse.bass as bass
import concourse.tile as tile
from concourse import mybir

F16 = mybir.dt.float16
F32 = mybir.dt.float32
AF = mybir.ActivationFunctionType
OP = mybir.AluOpType

MC = 131072
N_STEPS = 48
N_CORES = 8
MCL = MC // N_CORES          # 16384 samples per core
GF = 128                     # state grid: [128 partitions, GF free]
NQ = 4                       # quarters per step
QF = GF // NQ                # grid cols per quarter (32 -> 4096 samples)
QS = 128 * QF                # samples per quarter
CH = 512                     # matmul moving-dim chunk
PMW = 1024                   # psum tile free width (2 banks)
NSEG = QS // PMW             # psum tiles per (net, layer) stream (4)
CPS = PMW // CH              # matmul chunks per psum tile (2)

STRIKES_CALL = np.array([100., 105., 110., 115., 120., 125., 130., 135., 140., 145.], np.float32)
STRIKES_PUT = np.array([55., 60., 65., 70., 75., 80., 85., 90., 95., 100.], np.float32)


# ---------------------------------------------------------------------------
# Workaround: this walrus build accepts only ONE sync-wait command per
# instruction. Split any instruction with more waits into preceding
# same-engine Drain (ctrl no-op) instructions, one wait each — same-engine
# FIFO order makes this semantically identical.
def _split_sync_waits(bir_json: bytes) -> bytes:
    bir = orjson.loads(bir_json)
    for fn in bir.get("functions", []):
        for bb in fn.get("blocks", []):
            out = []
            changed = False
            for ins in bb.get("instructions", []):
                si = ins.get("sync_info") or {}
                waits = si.get("on_wait") or []
                if len(waits) > 1:
                    changed = True
                    for ci, w in enumerate(waits[:-1]):
                        out.append({
                            "name": f"{ins['name']}_sw{ci}",
                            "opcode": "Drain",
                            "engine": ins.get("engine", "SP"),
                            "ins": [], "outs": [],
                            "debug": ins.get("debug"),
                            "sync_info": {"on_update": [], "on_wait": [w]},
                        })
                    si["on_wait"] = waits[-1:]
                    ins["sync_info"] = si
                out.append(ins)
            if changed:
                bb["instructions"] = out
    return orjson.dumps(bir)


def _install_sync_split():
    import concourse.bass_utils as bu
    import concourse.bass2jax as b2j
    if getattr(bu, "_sync_split_installed", False):
        return
    orig = bu.compile_bir_kernel

    def patched(bir_json, tmpdir, neff_name="file.neff"):
        return orig(_split_sync_waits(bir_json), tmpdir, neff_name=neff_name)

    bu.compile_bir_kernel = patched
    bu._sync_split_installed = True
    if getattr(b2j, "compile_bir_kernel", None) is orig:
        b2j.compile_bir_kernel = patched


def build_nc(idx_steps, c0, bo0, bo1h, bo2, bo3, cS, cV, n_steps=N_STEPS,
             repeat=1):
    """Build the single-core Bass program (SPMD: all cores run the same code).

    idx_steps: list of 24 ints (step whose post-update S feeds output row i).
    c0 = 1 + r*h. bo* = output biases (net 1 pre-scaled by h).
    """
    nc = bass.Bass()

    z_in = nc.declare_dram_parameter("z", [n_steps, MCL], F32, isOutput=False)
    z1_in = nc.declare_dram_parameter("z1", [n_steps, MCL], F32, isOutput=False)
    wiT_in = nc.declare_dram_parameter("wiT", [128, 256], F16, isOutput=False)
    whT_in = nc.declare_dram_parameter("whT", [128, 1536], F16, isOutput=False)
    woT_in = nc.declare_dram_parameter("woT", [128, 16], F16, isOutput=False)
    b1_in = nc.declare_dram_parameter("b1", [128, 4 * n_steps], F32, isOutput=False)
    bh_in = nc.declare_dram_parameter("bh", [128, 12], F32, isOutput=False)
    strk_in = nc.declare_dram_parameter("strk", [128, 20], F32, isOutput=False)
    n_mat = len(idx_steps)
    acc_out = nc.declare_dram_parameter("acc", [128, 21 * n_mat], F32, isOutput=True)

    # maturity -> list of output rows (handles duplicate indices)
    mat_map = {}
    for i, st in enumerate(idx_steps):
        mat_map.setdefault(st, []).append(i)

    # SWDGE (Pool-issued) DMAs fail walrus codegen inside a For_i loop
    # ("ISA wrong length"), so the repeat timing build falls back to SP.
    ldma = None  # set below once nc engines exist

    with tile.TileContext(nc) as tc, ExitStack() as ctx:
        ldma = nc.gpsimd.dma_start if repeat == 1 else nc.sync.dma_start
        consts = ctx.enter_context(tc.tile_pool(name="consts", bufs=1))
        persist = ctx.enter_context(tc.tile_pool(name="persist", bufs=1))
        hpool = ctx.enter_context(tc.tile_pool(name="hpool", bufs=2))
        inppool = ctx.enter_context(tc.tile_pool(name="inppool", bufs=5))
        orowpool = ctx.enter_context(tc.tile_pool(name="orowpool", bufs=3))
        outspool = ctx.enter_context(tc.tile_pool(name="outspool", bufs=3))
        zpool = ctx.enter_context(tc.tile_pool(name="zpool", bufs=3))
        updpool = ctx.enter_context(tc.tile_pool(name="updpool", bufs=2))
        junkpool = ctx.enter_context(tc.tile_pool(name="junkpool", bufs=4))
        psmm = ctx.enter_context(tc.tile_pool(name="psmm", bufs=4, space="PSUM"))

        # ---- constants ----
        wiT = consts.tile([128, 256], F16)
        nc.sync.dma_start(out=wiT, in_=wiT_in[:, :])
        whT = consts.tile([128, 1536], F16)
        nc.sync.dma_start(out=whT, in_=whT_in[:, :])
        woT = consts.tile([128, 16], F16)
        nc.sync.dma_start(out=woT, in_=woT_in[:, :])
        b1 = consts.tile([128, 4 * n_steps], F32)
        nc.sync.dma_start(out=b1, in_=b1_in[:, :])
        bh = consts.tile([128, 12], F32)
        nc.sync.dma_start(out=bh, in_=bh_in[:, :])
        strk = consts.tile([128, 20], F32)
        nc.sync.dma_start(out=strk, in_=strk_in[:, :])

        # ---- persistent state ----
        S = persist.tile([128, GF], F32)
        V = persist.tile([128, GF], F32)
        S16 = persist.tile([128, GF], F16)
        V16 = persist.tile([128, GF], F16)
        acc = persist.tile([128, 21 * n_mat], F32)

        nc.vector.memset(S[:, :], cS)
        nc.vector.memset(V[:, :], cV)
        nc.vector.memset(S16[:, :], 0.0)
        nc.vector.memset(V16[:, :], 0.0)

        ua = updpool.tile([128, QF], F32, tag="ua")
        ub = updpool.tile([128, QF], F32, tag="ub")
        uc = updpool.tile([128, QF], F32, tag="uc")
        ud = updpool.tile([128, QF], F32, tag="ud")

        # drain engine alternation: DVE is slightly slower per drain, so it
        # takes 15 of every 32 big drains; payoff/small ops alternate evenly.
        state = {"k": 0, "pj": 0}

        def drain_relu(dst, pm_ap, bias_ap):
            k = state["k"]
            state["k"] += 1
            if (k * 15) // 32 != ((k + 1) * 15) // 32:
                nc.vector.tensor_scalar(out=dst, in0=pm_ap, scalar1=bias_ap,
                                        scalar2=0.0, op0=OP.add, op1=OP.max)
            else:
                nc.scalar.activation(dst, pm_ap, AF.Relu, bias=bias_ap, scale=1.0)

        def drain_copy(dst, pm_ap):
            k = state["k"]
            state["k"] += 1
            if (k * 15) // 32 != ((k + 1) * 15) // 32:
                nc.vector.tensor_copy(dst, pm_ap)
            else:
                nc.scalar.copy(dst, pm_ap)

        # initial first-layer input: centered state is exactly zero
        inp_tiles = []
        for q in range(NQ):
            t0 = inppool.tile([128, QS], F16, tag="inp")
            nc.vector.memset(t0[:, :], 0.0)
            inp_tiles.append(t0)

        def emit_payoff(row):
            # ACT only: DVE's accum_out does not produce free-dim sums on
            # this stack (verified empirically — it returns the last element).
            base = 21 * row
            for j in range(20):
                junk = junkpool.tile([128, GF], F32, tag="junk")
                col = acc[:, base + j:base + j + 1]
                nc.scalar.activation(junk, S, AF.Relu, bias=strk[:, j:j + 1],
                                     scale=1.0, accum_out=col)
            junk = junkpool.tile([128, GF], F32, tag="junk")
            nc.scalar.activation(junk, S, AF.Copy,
                                 accum_out=acc[:, base + 20:base + 21])

        # ---- main SDE loop (fully unrolled; repeat>1 is timing-only) ----
        rep_ctx = (tc.For_i(0, repeat, 1) if repeat > 1 else None)
        if rep_ctx is not None:
            rep_ctx.__enter__()
            # reset state each iteration so every pass computes the same
            # finite values (garbage/NaN state would distort engine timing)
            nc.vector.memset(S[:, :], cS)
            nc.vector.memset(V[:, :], cV)
            nc.vector.memset(S16[:, :], 0.0)
            nc.vector.memset(V16[:, :], 0.0)
            for q in range(NQ):
                rt = inppool.tile([128, QS], F16, tag="inp")
                nc.vector.memset(rt[:, :], 0.0)
                inp_tiles[q] = rt
        for t in range(n_steps):
            z_t = zpool.tile([128, GF], F32, tag="z")
            ldma(out=z_t[:, :], in_=z_in[t:t + 1, :])
            z1_t = zpool.tile([128, GF], F32, tag="z1")
            ldma(out=z1_t[:, :], in_=z1_in[t:t + 1, :])

            for q in range(NQ):
                qs = slice(QF * q, QF * q + QF)
                inp_q = inp_tiles[q]
                # -- first layer: 2-way row-tiled K=2 (strips 0 and 32);
                # net pairs (0,1) then (2,3) run concurrently on the PE with
                # their two psum tiles draining on DVE and ACT in parallel --
                hcur = [None] * 4
                for na, nb in ((0, 1), (2, 3)):
                    ha = hpool.tile([128, QS], F16, tag=f"h{na}")
                    hb = hpool.tile([128, QS], F16, tag=f"h{nb}")
                    for seg in range(NSEG):
                        pma = psmm.tile([128, PMW], F32, tag="pm")
                        pmb = psmm.tile([128, PMW], F32, tag="pm")
                        for c in range(CPS):
                            col = seg * PMW + c * CH
                            for n, pm in ((na, pma), (nb, pmb)):
                                strip = 32 * (n % 2)
                                wcol = 128 * (n // 2)
                                nc.tensor.matmul(
                                    pm[:, c * CH:(c + 1) * CH],
                                    lhsT=wiT[strip:strip + 2, wcol:wcol + 128],
                                    rhs=inp_q[strip:strip + 2, col:col + CH],
                                    start=True, stop=True, tile_position=(strip, 0))
                        for n, pm, hn in ((na, pma, ha), (nb, pmb, hb)):
                            drain_relu(hn[:, seg * PMW:(seg + 1) * PMW], pm[:, :],
                                       b1[:, 4 * t + n:4 * t + n + 1])
                    hcur[na] = ha
                    hcur[nb] = hb
                # -- hidden layers --
                for l in range(3):
                    for n in range(4):
                        hn = hpool.tile([128, QS], F16, tag=f"h{n}")
                        w_sl = slice((n * 3 + l) * 128, (n * 3 + l + 1) * 128)
                        for seg in range(NSEG):
                            pm = psmm.tile([128, PMW], F32, tag="pm")
                            for c in range(CPS):
                                col = seg * PMW + c * CH
                                nc.tensor.matmul(
                                    pm[:, c * CH:(c + 1) * CH],
                                    lhsT=whT[:, w_sl],
                                    rhs=hcur[n][:, col:col + CH],
                                    start=True, stop=True)
                            drain_relu(hn[:, seg * PMW:(seg + 1) * PMW], pm[:, :],
                                       bh[:, n * 3 + l:n * 3 + l + 1])
                        hcur[n] = hn
                # -- output layer: M=4 zero-padded weights, PSUM-accumulated
                # across the 4 nets so outputs land on partitions 0-3 --
                orow = orowpool.tile([4, QS], F32, tag="orow")
                for seg in range(NSEG):
                    pmo = psmm.tile([128, PMW], F32, tag="pm")
                    for c in range(CPS):
                        for n in range(4):
                            col = seg * PMW + c * CH
                            nc.tensor.matmul(
                                pmo[0:4, c * CH:(c + 1) * CH],
                                lhsT=woT[:, 4 * n:4 * n + 4],
                                rhs=hcur[n][:, col:col + CH],
                                start=(n == 0), stop=(n == 3))
                    drain_copy(orow[0:4, seg * PMW:(seg + 1) * PMW],
                               pmo[0:4, :])
                # -- scatter net outputs back to the state grid --
                og = []
                for n in range(4):
                    g = outspool.tile([128, QF], F32, tag=f"og{n}")
                    ldma(out=g[:, :], in_=orow[n:n + 1, :])
                    og.append(g)
                # -- state update (fp32, DVE) --
                Ssl = S[:, qs]
                Vsl = V[:, qs]
                zsl = z_t[:, qs]
                z1sl = z1_t[:, qs]
                # state update: scalar_tensor_tensor must stay on DVE (walrus
                # rejects it on Pool); plain tensor ops go to the idle GPSIMD
                # S' = relu(c0*S + (diff+bo0)*dW)
                nc.vector.scalar_tensor_tensor(out=ua, in0=og[0], scalar=bo0,
                                               in1=zsl, op0=OP.add, op1=OP.mult)
                nc.vector.scalar_tensor_tensor(out=ub, in0=Ssl, scalar=c0,
                                               in1=ua, op0=OP.mult, op1=OP.add)
                nc.vector.tensor_scalar(out=Ssl, in0=ub, scalar1=0.0,
                                        scalar2=None, op0=OP.max)
                # V' = V + (driftV*h+bo1h) + (diffV+bo2)*dW + (diffV1+bo3)*dW1
                nc.vector.scalar_tensor_tensor(out=ua, in0=og[1], scalar=bo1h,
                                               in1=Vsl, op0=OP.add, op1=OP.add)
                nc.vector.scalar_tensor_tensor(out=ub, in0=og[2], scalar=bo2,
                                               in1=zsl, op0=OP.add, op1=OP.mult)
                nc.vector.scalar_tensor_tensor(out=uc, in0=og[3], scalar=bo3,
                                               in1=z1sl, op0=OP.add, op1=OP.mult)
                nc.vector.tensor_tensor(out=ud, in0=ua, in1=ub, op=OP.add)
                nc.vector.tensor_tensor(out=Vsl, in0=ud, in1=uc, op=OP.add)
                # centered fp16 copies for the next step's first layer
                nc.vector.tensor_scalar(out=S16[:, qs], in0=Ssl, scalar1=cS,
                                        scalar2=None, op0=OP.subtract)
                nc.vector.tensor_scalar(out=V16[:, qs], in0=Vsl, scalar1=cV,
                                        scalar2=None, op0=OP.subtract)
                # build next step's first-layer input rows (4 replicas for
                # the row-tiled first layer)
                if t + 1 < n_steps or repeat > 1:
                    ninp = inppool.tile([128, QS], F16, tag="inp")
                    for strip in (0, 32):
                        ldma(out=ninp[strip:strip + 1, :], in_=S16[:, qs])
                        ldma(out=ninp[strip + 1:strip + 2, :], in_=V16[:, qs])
                    inp_tiles[q] = ninp

            # payoffs for maturities at this step (post-update S)
            for row in mat_map.get(t, []):
                emit_payoff(row)
        if rep_ctx is not None:
            rep_ctx.__exit__(None, None, None)

        nc.sync.dma_start(out=acc_out[:, :], in_=acc)

    return nc


def _prep_inputs(S0, V0, rate, z, z1, indices, timegrid, Wi, bi, Wh, bh, Wo, bo,
                 n_steps=None):
    """Host-side preprocessing. Returns (build args, per-core inputs, disc)."""
    S0v = float(np.asarray(S0).reshape(-1)[0])
    V0v = float(np.asarray(V0).reshape(-1)[0])
    r = float(np.asarray(rate).reshape(-1)[0])
    z = np.asarray(z, np.float32)
    z1 = np.asarray(z1, np.float32)
    if n_steps is None:
        n_steps = z.shape[1]
    tg = np.asarray(timegrid, np.float64)
    h = float(tg[1] - tg[0])
    sqh = float(np.sqrt(h))
    c0 = 1.0 + r * h

    Wi = np.asarray(Wi, np.float32)
    bi = np.asarray(bi, np.float32)
    Wh = np.asarray(Wh, np.float32)
    bhv = np.asarray(bh, np.float32)
    Wo = np.asarray(Wo, np.float32).copy()
    bo = np.asarray(bo, np.float32).copy()
    # driftV net (index 1) is only ever used multiplied by h -> fold h into it
    Wo[1] *= h
    bo0, bo1h, bo2, bo3 = (float(bo[0, 0]), float(bo[1, 0]) * h,
                           float(bo[2, 0]), float(bo[3, 0]))

    cS, cV = S0v, V0v    # centering constants for fp16 inputs
    # first-layer bias with t-term and centering folded in: [4, n_steps, 128]
    t_vals = tg[:n_steps].astype(np.float32)
    b1 = (bi[:, None, :] + t_vals[None, :, None] * Wi[:, 0][:, None, :]
          + cS * Wi[:, 1][:, None, :] + cV * Wi[:, 2][:, None, :])
    # device layout: [128 features, n_steps*4] with col t*4+n
    b1_dev = np.ascontiguousarray(b1.transpose(2, 1, 0).reshape(128, n_steps * 4),
                                  np.float32)

    # first-layer weights, 2-way row strips: net n lives at partition rows
    # 32*(n%2).. and weight cols 128*(n//2)..
    wiT_dev = np.zeros((128, 256), np.float16)
    for n in range(4):
        strip, wcol = 32 * (n % 2), 128 * (n // 2)
        wiT_dev[strip:strip + 2, wcol:wcol + 128] = Wi[n, 1:3, :].astype(np.float16)
    whT_dev = np.ascontiguousarray(
        Wh.transpose(2, 0, 1, 3).reshape(128, 12 * 128), np.float16)
    # output weights, zero-padded to M=4 per net: cols 4n..4n+4, col 4n+j
    # is Wo[n] when j == n else 0 (outputs accumulate on psum partitions 0-3)
    woT_dev = np.zeros((128, 16), np.float16)
    for n in range(4):
        woT_dev[:, 4 * n + n] = Wo[n, :, 0].astype(np.float16)
    bh_dev = np.ascontiguousarray(bhv.transpose(2, 0, 1).reshape(128, 12), np.float32)

    strk_dev = np.ascontiguousarray(
        np.tile(np.concatenate([-STRIKES_CALL, -STRIKES_PUT])[None, :], (128, 1)),
        np.float32)

    idx = np.asarray(indices).astype(np.int64).reshape(-1)
    idx_steps = [int((v - 1) % n_steps) for v in idx]
    disc = np.exp(-r * 2.0 * idx.astype(np.float64) / n_steps)

    in_maps = []
    for k in range(N_CORES):
        sl = slice(k * MCL, (k + 1) * MCL)
        in_maps.append({
            "z": np.ascontiguousarray((z[sl, :n_steps] * sqh).T, np.float32),
            "z1": np.ascontiguousarray((z1[sl, :n_steps] * sqh).T, np.float32),
            "wiT": wiT_dev, "whT": whT_dev, "woT": woT_dev,
            "b1": b1_dev, "bh": bh_dev, "strk": strk_dev,
        })
    build_args = dict(idx_steps=idx_steps, c0=c0, bo0=bo0, bo1h=bo1h,
                      bo2=bo2, bo3=bo3, cS=cS, cV=cV, n_steps=n_steps)
    return build_args, in_maps, disc


def _combine(results, disc, idx_steps):
    """Sum per-core [128, 21*n_mat] partials into the [96, 10] output."""
    n_mat = len(idx_steps)
    total = np.zeros((128, 21 * n_mat), np.float64)
    for res in results:
        total += np.asarray(res["acc"], np.float64)
    cols = total.sum(axis=0).reshape(n_mat, 21)
    calls_c = cols[:, 0:10]                  # sum relu(S - Kc)
    calls_p = cols[:, 10:20]                 # sum relu(S - Kp)
    sumS = cols[:, 20:21]                    # sum S
    kc = STRIKES_CALL.astype(np.float64)[None, :]
    kp = STRIKES_PUT.astype(np.float64)[None, :]
    # relu(K - S) = relu(S - K) - S + K  (summed over MC samples)
    puts_c = calls_c - sumS + MC * kc
    puts_p = calls_p - sumS + MC * kp
    out = np.concatenate([calls_c, puts_p, calls_p, puts_c], axis=0) / MC
    out = out * np.concatenate([disc] * 4)[:, None]
    return out.astype(np.float32)


def kernel(**inputs) -> np.ndarray:
    from concourse.bass_utils import run_bass_kernel_spmd
    _install_sync_split()
    build_args, in_maps, disc = _prep_inputs(**inputs)
    nc = build_nc(**build_args)
    res = run_bass_kernel_spmd(nc, in_maps, list(range(N_CORES)))
    return _combine(res.results, disc, build_args["idx_steps"])


# revision 31
# speedup vs baseline: 4.0681x; 1.0611x over previous
"""Trainium2 Bass kernel for nn_Net_SDE: 48-step neural SDE Monte-Carlo pricer.

Data-parallel over the 131072 MC samples across 8 NeuronCores (16384/core).

Per core the 48 SDE steps run as a For_i loop over NI=2 iterations of a
24-step unrolled body (measured sweet spot: a 73K-instruction fully unrolled
program runs ~60% slower per step than a 37K-instruction body looped twice,
while small bodies pay a large per-back-edge cost).  Each step is processed
in four 4096-sample "quarters" so that the inter-step boundary work (state
update, layout shuffles) of quarter q hides under the MLP streams of
quarter q+1.

Within a quarter, the 4 MLPs are evaluated layer-wise as weight-stationary
streams of 8 N=512 fp16 matmuls into four [128, 1024] (2-bank) PSUM tiles;
each PSUM tile is drained by one FD=1024 bias+relu op, alternating between
the vector (DVE) and scalar (ACT) engines — the two drain engines together
are the throughput limit of the whole kernel.  The first layer is 2-way
row-tiled (tile_position=(0|32, 0), K=2) and the output layer uses M=4
zero-padded weights PSUM-accumulated across the 4 nets so all outputs land
on contiguous psum partitions 0-3.

Payoffs are computed after every maturity step into per-(iteration, step)
DRAM slots (uniform body; the host gathers slots per the runtime `indices`).
Only call partials relu(S-K) and sum(S) are computed on-device (21 ops per
step instead of 40); the host derives puts exactly via put-call parity:
relu(K-S) = relu(S-K) - (S-K).
"""
import numpy as np
from contextlib import ExitStack

import orjson

import concourse.bass as bass
import concourse.tile as tile
from concourse import mybir

F16 = mybir.dt.float16
F32 = mybir.dt.float32
AF = mybir.ActivationFunctionType
OP = mybir.AluOpType

MC = 131072
N_STEPS = 48
N_CORES = 8
MCL = MC // N_CORES          # 16384 samples per core
GF = 128                     # state grid: [128 partitions, GF free]
NQ = 4                       # quarters per step
QF = GF // NQ                # grid cols per quarter (32 -> 4096 samples)
QS = 128 * QF                # samples per quarter
CH = 512                     # matmul moving-dim chunk
PMW = 1024                   # psum tile free width (2 banks)
NSEG = QS // PMW             # psum tiles per (net, layer) stream (4)
CPS = PMW // CH              # matmul chunks per psum tile (2)
U_STEPS = 24                 # unrolled steps per For_i iteration

STRIKES_CALL = np.array([100., 105., 110., 115., 120., 125., 130., 135., 140., 145.], np.float32)
STRIKES_PUT = np.array([55., 60., 65., 70., 75., 80., 85., 90., 95., 100.], np.float32)


# ---------------------------------------------------------------------------
# Workaround: this walrus build accepts only ONE sync-wait command per
# instruction. Split any instruction with more waits into preceding
# same-engine Drain (ctrl no-op) instructions, one wait each — same-engine
# FIFO order makes this semantically identical.
def _split_sync_waits(bir_json: bytes) -> bytes:
    bir = orjson.loads(bir_json)
    for fn in bir.get("functions", []):
        for bb in fn.get("blocks", []):
            out = []
            changed = False
            for ins in bb.get("instructions", []):
                si = ins.get("sync_info") or {}
                waits = si.get("on_wait") or []
                if len(waits) > 1:
                    changed = True
                    for ci, w in enumerate(waits[:-1]):
                        out.append({
                            "name": f"{ins['name']}_sw{ci}",
                            "opcode": "Drain",
                            "engine": ins.get("engine", "SP"),
                            "ins": [], "outs": [],
                            "debug": ins.get("debug"),
                            "sync_info": {"on_update": [], "on_wait": [w]},
                        })
                    si["on_wait"] = waits[-1:]
                    ins["sync_info"] = si
                out.append(ins)
            if changed:
                bb["instructions"] = out
    return orjson.dumps(bir)


def _install_sync_split():
    import concourse.bass_utils as bu
    import concourse.bass2jax as b2j
    if getattr(bu, "_sync_split_installed", False):
        return
    orig = bu.compile_bir_kernel

    def patched(bir_json, tmpdir, neff_name="file.neff"):
        return orig(_split_sync_waits(bir_json), tmpdir, neff_name=neff_name)

    bu.compile_bir_kernel = patched
    bu._sync_split_installed = True
    if getattr(b2j, "compile_bir_kernel", None) is orig:
        b2j.compile_bir_kernel = patched


def build_nc(payoff_us, c0, bo0, bo1h, bo2, bo3, cS, cV, n_steps=N_STEPS,
             u_steps=None, repeat=1):
    """Build the single-core Bass program (SPMD: all cores run the same code).

    payoff_us: sorted in-body step offsets u that need payoff slots (union
    over iterations). c0 = 1 + r*h. bo* = output biases (net 1 pre-scaled
    by h).
    """
    if u_steps is None:
        u_steps = U_STEPS if n_steps % U_STEPS == 0 else n_steps
    assert n_steps % u_steps == 0
    NI = n_steps // u_steps
    U = u_steps

    nc = bass.Bass()

    z_in = nc.declare_dram_parameter("z", [NI, U * MCL], F32, isOutput=False)
    z1_in = nc.declare_dram_parameter("z1", [NI, U * MCL], F32, isOutput=False)
    wiT_in = nc.declare_dram_parameter("wiT", [128, 256], F16, isOutput=False)
    whT_in = nc.declare_dram_parameter("whT", [128, 1536], F16, isOutput=False)
    woT_in = nc.declare_dram_parameter("woT", [128, 16], F16, isOutput=False)
    b1_in = nc.declare_dram_parameter("b1", [NI, 128, 4 * U], F32, isOutput=False)
    bh_in = nc.declare_dram_parameter("bh", [128, 12], F32, isOutput=False)
    strk_in = nc.declare_dram_parameter("strk", [128, 20], F32, isOutput=False)
    payoff_us = sorted(set(payoff_us))
    n_slots = max(1, len(payoff_us))
    slot_of = {u: i for i, u in enumerate(payoff_us)}
    acc_out = nc.declare_dram_parameter("acc", [NI, 128, 21 * n_slots], F32,
                                        isOutput=True)

    with tile.TileContext(nc) as tc, ExitStack() as ctx:
        consts = ctx.enter_context(tc.tile_pool(name="consts", bufs=1))
        persist = ctx.enter_context(tc.tile_pool(name="persist", bufs=1))
        hpool = ctx.enter_context(tc.tile_pool(name="hpool", bufs=2))
        inppool = ctx.enter_context(tc.tile_pool(name="inppool", bufs=5))
        orowpool = ctx.enter_context(tc.tile_pool(name="orowpool", bufs=2))
        outspool = ctx.enter_context(tc.tile_pool(name="outspool", bufs=3))
        zpool = ctx.enter_context(tc.tile_pool(name="zpool", bufs=1))
        updpool = ctx.enter_context(tc.tile_pool(name="updpool", bufs=2))
        junkpool = ctx.enter_context(tc.tile_pool(name="junkpool", bufs=4))
        psmm = ctx.enter_context(tc.tile_pool(name="psmm", bufs=4, space="PSUM"))

        # ---- constants ----
        wiT = consts.tile([128, 256], F16)
        nc.sync.dma_start(out=wiT, in_=wiT_in[:, :])
        whT = consts.tile([128, 1536], F16)
        nc.sync.dma_start(out=whT, in_=whT_in[:, :])
        woT = consts.tile([128, 16], F16)
        nc.sync.dma_start(out=woT, in_=woT_in[:, :])
        bh = consts.tile([128, 12], F32)
        nc.sync.dma_start(out=bh, in_=bh_in[:, :])
        strk = consts.tile([128, 20], F32)
        nc.sync.dma_start(out=strk, in_=strk_in[:, :])

        # ---- persistent state ----
        S = persist.tile([128, GF], F32)
        V = persist.tile([128, GF], F32)
        S16 = persist.tile([128, GF], F16)
        V16 = persist.tile([128, GF], F16)
        # step-0 first-layer inputs are persistent: the body's last step
        # writes them for the next For_i iteration to read
        inp0_0 = persist.tile([128, QS], F16)
        inp0_1 = persist.tile([128, QS], F16)
        inp0_2 = persist.tile([128, QS], F16)
        inp0_3 = persist.tile([128, QS], F16)
        inp0 = [inp0_0, inp0_1, inp0_2, inp0_3]
        acc_all = persist.tile([128, 21 * n_slots], F32)

        def reset_state():
            nc.vector.memset(S[:, :], cS)
            nc.vector.memset(V[:, :], cV)
            nc.vector.memset(S16[:, :], 0.0)
            nc.vector.memset(V16[:, :], 0.0)
            for q in range(NQ):
                nc.vector.memset(inp0[q][:, :], 0.0)

        reset_state()

        ua = updpool.tile([128, QF], F32, tag="ua")
        ub = updpool.tile([128, QF], F32, tag="ub")
        uc = updpool.tile([128, QF], F32, tag="uc")
        ud = updpool.tile([128, QF], F32, tag="ud")

        # drain engine alternation: DVE is slightly slower per drain, so it
        # takes 15 of every 32 drains.
        state = {"k": 0}

        def drain_relu(dst, pm_ap, bias_ap):
            k = state["k"]
            state["k"] += 1
            if (k * 15) // 32 != ((k + 1) * 15) // 32:
                nc.vector.tensor_scalar(out=dst, in0=pm_ap, scalar1=bias_ap,
                                        scalar2=0.0, op0=OP.add, op1=OP.max)
            else:
                nc.scalar.activation(dst, pm_ap, AF.Relu, bias=bias_ap, scale=1.0)

        def drain_copy(dst, pm_ap):
            k = state["k"]
            state["k"] += 1
            if (k * 15) // 32 != ((k + 1) * 15) // 32:
                nc.vector.tensor_copy(dst, pm_ap)
            else:
                nc.scalar.copy(dst, pm_ap)

        def emit_payoff(u):
            # 20x relu(S-K) partials + sum(S) into this step's acc_all slot.
            # ACT only: DVE's accum_out does not produce free-dim sums on
            # this stack (verified empirically — it returns the last value).
            base = 21 * slot_of[u]
            for j in range(20):
                junk = junkpool.tile([128, GF], F32, tag="junk")
                nc.scalar.activation(junk, S, AF.Relu, bias=strk[:, j:j + 1],
                                     scale=1.0, accum_out=acc_all[:, base + j:base + j + 1])
            junk = junkpool.tile([128, GF], F32, tag="junk")
            nc.scalar.activation(junk, S, AF.Copy,
                                 accum_out=acc_all[:, base + 20:base + 21])

        def sde_body(iv):
            inp_tiles = list(inp0)
            # per-iteration parameter loads (register-indexed DMAs; only a
            # handful per body — many bounds-checked dynamic DMAs per loop
            # break walrus AP lowering)
            b1 = zpool.tile([128, 4 * U], F32, tag="b1")
            nc.sync.dma_start(out=b1[:, :], in_=b1_in[bass.ds(iv, 1), :, :])
            z_all = zpool.tile([128, U * GF], F32, tag="zall")
            nc.sync.dma_start(out=z_all[:, :], in_=z_in[bass.ds(iv, 1), :])
            z1_all = zpool.tile([128, U * GF], F32, tag="z1all")
            nc.sync.dma_start(out=z1_all[:, :], in_=z1_in[bass.ds(iv, 1), :])
            for u in range(U):

                for q in range(NQ):
                    qs = slice(QF * q, QF * q + QF)
                    inp_q = inp_tiles[q]
                    # -- first layer: 2-way row-tiled K=2 (strips 0 and 32);
                    # net pairs (0,1) then (2,3) run concurrently on the PE,
                    # their psum tiles draining on DVE and ACT in parallel --
                    hcur = [None] * 4
                    for na, nb in ((0, 1), (2, 3)):
                        ha = hpool.tile([128, QS], F16, tag=f"h{na}")
                        hb = hpool.tile([128, QS], F16, tag=f"h{nb}")
                        for seg in range(NSEG):
                            pma = psmm.tile([128, PMW], F32, tag="pm")
                            pmb = psmm.tile([128, PMW], F32, tag="pm")
                            for c in range(CPS):
                                col = seg * PMW + c * CH
                                for n, pm in ((na, pma), (nb, pmb)):
                                    strip = 32 * (n % 2)
                                    wcol = 128 * (n // 2)
                                    nc.tensor.matmul(
                                        pm[:, c * CH:(c + 1) * CH],
                                        lhsT=wiT[strip:strip + 2, wcol:wcol + 128],
                                        rhs=inp_q[strip:strip + 2, col:col + CH],
                                        start=True, stop=True,
                                        tile_position=(strip, 0))
                            for n, pm, hn in ((na, pma, ha), (nb, pmb, hb)):
                                drain_relu(hn[:, seg * PMW:(seg + 1) * PMW],
                                           pm[:, :], b1[:, 4 * u + n:4 * u + n + 1])
                        hcur[na] = ha
                        hcur[nb] = hb
                    # -- hidden layers --
                    for l in range(3):
                        for n in range(4):
                            hn = hpool.tile([128, QS], F16, tag=f"h{n}")
                            w_sl = slice((n * 3 + l) * 128, (n * 3 + l + 1) * 128)
                            for seg in range(NSEG):
                                pm = psmm.tile([128, PMW], F32, tag="pm")
                                for c in range(CPS):
                                    col = seg * PMW + c * CH
                                    nc.tensor.matmul(
                                        pm[:, c * CH:(c + 1) * CH],
                                        lhsT=whT[:, w_sl],
                                        rhs=hcur[n][:, col:col + CH],
                                        start=True, stop=True)
                                drain_relu(hn[:, seg * PMW:(seg + 1) * PMW],
                                           pm[:, :], bh[:, n * 3 + l:n * 3 + l + 1])
                            hcur[n] = hn
                    # -- output layer: M=4 zero-padded weights, accumulated
                    # across the 4 nets so outputs land on partitions 0-3 --
                    orow = orowpool.tile([4, QS], F32, tag="orow")
                    for seg in range(NSEG):
                        pmo = psmm.tile([128, PMW], F32, tag="pm")
                        for c in range(CPS):
                            for n in range(4):
                                col = seg * PMW + c * CH
                                nc.tensor.matmul(
                                    pmo[0:4, c * CH:(c + 1) * CH],
                                    lhsT=woT[:, 4 * n:4 * n + 4],
                                    rhs=hcur[n][:, col:col + CH],
                                    start=(n == 0), stop=(n == 3))
                        drain_copy(orow[0:4, seg * PMW:(seg + 1) * PMW],
                                   pmo[0:4, :])
                    # -- scatter net outputs back to the state grid --
                    og = []
                    for n in range(4):
                        g = outspool.tile([128, QF], F32, tag=f"og{n}")
                        nc.sync.dma_start(out=g[:, :], in_=orow[n:n + 1, :])
                        og.append(g)
                    # -- state update (fp32, DVE) --
                    Ssl = S[:, qs]
                    Vsl = V[:, qs]
                    zsl = z_all[:, u * GF + QF * q:u * GF + QF * q + QF]
                    z1sl = z1_all[:, u * GF + QF * q:u * GF + QF * q + QF]
                    # S' = relu(c0*S + (diff+bo0)*dW)
                    nc.vector.scalar_tensor_tensor(out=ua, in0=og[0], scalar=bo0,
                                                   in1=zsl, op0=OP.add, op1=OP.mult)
                    nc.vector.scalar_tensor_tensor(out=ub, in0=Ssl, scalar=c0,
                                                   in1=ua, op0=OP.mult, op1=OP.add)
                    nc.vector.tensor_scalar(out=Ssl, in0=ub, scalar1=0.0,
                                            scalar2=None, op0=OP.max)
                    # V' = V + (driftV*h+bo1h) + (diffV+bo2)*dW + (diffV1+bo3)*dW1
                    nc.vector.scalar_tensor_tensor(out=ua, in0=og[1], scalar=bo1h,
                                                   in1=Vsl, op0=OP.add, op1=OP.add)
                    nc.vector.scalar_tensor_tensor(out=ub, in0=og[2], scalar=bo2,
                                                   in1=zsl, op0=OP.add, op1=OP.mult)
                    nc.vector.scalar_tensor_tensor(out=uc, in0=og[3], scalar=bo3,
                                                   in1=z1sl, op0=OP.add, op1=OP.mult)
                    nc.vector.tensor_tensor(out=ud, in0=ua, in1=ub, op=OP.add)
                    nc.vector.tensor_tensor(out=Vsl, in0=ud, in1=uc, op=OP.add)
                    # centered fp16 copies for the next step's first layer
                    nc.vector.tensor_scalar(out=S16[:, qs], in0=Ssl, scalar1=cS,
                                            scalar2=None, op0=OP.subtract)
                    nc.vector.tensor_scalar(out=V16[:, qs], in0=Vsl, scalar1=cV,
                                            scalar2=None, op0=OP.subtract)
                    # next step's first-layer input rows (2 replicas for the
                    # 2-way row-tiled first layer); the last step writes the
                    # persistent tiles read by the next For_i iteration
                    if u + 1 < U:
                        ninp = inppool.tile([128, QS], F16, tag="inp")
                    else:
                        ninp = inp0[q]
                    for strip in (0, 32):
                        nc.sync.dma_start(out=ninp[strip:strip + 1, :],
                                          in_=S16[:, qs])
                        nc.sync.dma_start(out=ninp[strip + 1:strip + 2, :],
                                          in_=V16[:, qs])
                    inp_tiles[q] = ninp

                if u in payoff_us:
                    emit_payoff(u)
            nc.sync.dma_start(out=acc_out[bass.ds(iv, 1), :, :], in_=acc_all)

        rep_ctx = (tc.For_i(0, repeat, 1) if repeat > 1 else None)
        if rep_ctx is not None:
            rep_ctx.__enter__()
            reset_state()
        with tc.For_i(0, NI, 1) as iv:
            sde_body(iv)
        if rep_ctx is not None:
            rep_ctx.__exit__(None, None, None)

    return nc


def _prep_inputs(S0, V0, rate, z, z1, indices, timegrid, Wi, bi, Wh, bh, Wo, bo,
                 n_steps=None, u_steps=None):
    """Host-side preprocessing. Returns (build args, per-core inputs, disc,
    idx_steps)."""
    S0v = float(np.asarray(S0).reshape(-1)[0])
    V0v = float(np.asarray(V0).reshape(-1)[0])
    r = float(np.asarray(rate).reshape(-1)[0])
    z = np.asarray(z, np.float32)
    z1 = np.asarray(z1, np.float32)
    if n_steps is None:
        n_steps = z.shape[1]
    if u_steps is None:
        u_steps = U_STEPS if n_steps % U_STEPS == 0 else n_steps
    NI = n_steps // u_steps
    U = u_steps
    tg = np.asarray(timegrid, np.float64)
    h = float(tg[1] - tg[0])
    sqh = float(np.sqrt(h))
    c0 = 1.0 + r * h

    Wi = np.asarray(Wi, np.float32)
    bi = np.asarray(bi, np.float32)
    Wh = np.asarray(Wh, np.float32)
    bhv = np.asarray(bh, np.float32)
    Wo = np.asarray(Wo, np.float32).copy()
    bo = np.asarray(bo, np.float32).copy()
    # driftV net (index 1) is only ever used multiplied by h -> fold h into it
    Wo[1] *= h
    bo0, bo1h, bo2, bo3 = (float(bo[0, 0]), float(bo[1, 0]) * h,
                           float(bo[2, 0]), float(bo[3, 0]))

    cS, cV = S0v, V0v    # centering constants for fp16 inputs
    # first-layer bias with t-term and centering folded in: [4, n_steps, 128]
    t_vals = tg[:n_steps].astype(np.float32)
    b1 = (bi[:, None, :] + t_vals[None, :, None] * Wi[:, 0][:, None, :]
          + cS * Wi[:, 1][:, None, :] + cV * Wi[:, 2][:, None, :])
    # device layout: [NI, 128, 4U] with slab i, col u*4+n for step t = i*U+u
    b1_f_t_n = b1.transpose(2, 1, 0)                     # [128, n_steps, 4]
    b1_dev = np.ascontiguousarray(
        b1_f_t_n.reshape(128, NI, 4 * U).transpose(1, 0, 2), np.float32)

    # first-layer weights, 2-way row strips: net n lives at partition rows
    # 32*(n%2).. and weight cols 128*(n//2)..
    wiT_dev = np.zeros((128, 256), np.float16)
    for n in range(4):
        strip, wcol = 32 * (n % 2), 128 * (n // 2)
        wiT_dev[strip:strip + 2, wcol:wcol + 128] = Wi[n, 1:3, :].astype(np.float16)
    whT_dev = np.ascontiguousarray(
        Wh.transpose(2, 0, 1, 3).reshape(128, 12 * 128), np.float16)
    # output weights, zero-padded to M=4 per net: col 4n+j is Wo[n] when
    # j == n else 0 (outputs accumulate on psum partitions 0-3)
    woT_dev = np.zeros((128, 16), np.float16)
    for n in range(4):
        woT_dev[:, 4 * n + n] = Wo[n, :, 0].astype(np.float16)
    bh_dev = np.ascontiguousarray(bhv.transpose(2, 0, 1).reshape(128, 12), np.float32)

    strk_dev = np.ascontiguousarray(
        np.tile(np.concatenate([-STRIKES_CALL, -STRIKES_PUT])[None, :], (128, 1)),
        np.float32)

    idx = np.asarray(indices).astype(np.int64).reshape(-1)
    idx_steps = [int((v - 1) % n_steps) for v in idx]
    payoff_us = sorted({st % U for st in idx_steps})
    disc = np.exp(-r * 2.0 * idx.astype(np.float64) / n_steps)

    def z_layout(zc):
        # [MCL, n_steps] -> [NI, p, u, f] so that per-step grid slices of the
        # SBUF slab are static: slab[p, u*128 + f] = z[p*128+f, iv*U+u]
        a = zc.reshape(128, 128, NI, U)          # [p, f, iv, u]
        return np.ascontiguousarray(
            a.transpose(2, 0, 3, 1).reshape(NI, U * MCL), np.float32)

    in_maps = []
    for k in range(N_CORES):
        sl = slice(k * MCL, (k + 1) * MCL)
        zt = (z[sl, :n_steps] * sqh).astype(np.float32)
        z1t = (z1[sl, :n_steps] * sqh).astype(np.float32)
        in_maps.append({
            "z": z_layout(zt),
            "z1": z_layout(z1t),
            "wiT": wiT_dev, "whT": whT_dev, "woT": woT_dev,
            "b1": b1_dev, "bh": bh_dev, "strk": strk_dev,
        })
    build_args = dict(payoff_us=payoff_us, c0=c0, bo0=bo0, bo1h=bo1h,
                      bo2=bo2, bo3=bo3, cS=cS, cV=cV, n_steps=n_steps,
                      u_steps=U)
    return build_args, in_maps, disc, idx_steps


def _combine(results, disc, idx_steps, n_steps, u_steps):
    """Gather per-step slots into the [96, 10] output via put-call parity."""
    U = u_steps
    NI = n_steps // U
    payoff_us = sorted({st % U for st in idx_steps})
    slot_of = {u: i for i, u in enumerate(payoff_us)}
    n_slots = max(1, len(payoff_us))
    total = np.zeros((NI, 128, 21 * n_slots), np.float64)
    for res in results:
        total += np.asarray(res["acc"], np.float64)
    cols = total.sum(axis=1).reshape(NI, n_slots, 21)
    n_mat = len(idx_steps)
    calls_c = np.zeros((n_mat, 10))
    calls_p = np.zeros((n_mat, 10))
    sumS = np.zeros((n_mat, 1))
    for m, st in enumerate(idx_steps):
        row = cols[st // U, slot_of[st % U]]
        calls_c[m] = row[0:10]
        calls_p[m] = row[10:20]
        sumS[m, 0] = row[20]
    kc = STRIKES_CALL.astype(np.float64)[None, :]
    kp = STRIKES_PUT.astype(np.float64)[None, :]
    # relu(K - S) = relu(S - K) - S + K  (summed over MC samples)
    puts_c = calls_c - sumS + MC * kc
    puts_p = calls_p - sumS + MC * kp
    out = np.concatenate([calls_c, puts_p, calls_p, puts_c], axis=0) / MC
    out = out * np.concatenate([disc] * 4)[:, None]
    return out.astype(np.float32)


def kernel(**inputs) -> np.ndarray:
    from concourse.bass_utils import run_bass_kernel_spmd
    _install_sync_split()
    build_args, in_maps, disc, idx_steps = _prep_inputs(**inputs)
    nc = build_nc(**build_args)
    res = run_bass_kernel_spmd(nc, in_maps, list(range(N_CORES)))
    return _combine(res.results, disc, idx_steps, build_args["n_steps"],
                    build_args["u_steps"])


# revision 32
# speedup vs baseline: 4.6578x; 1.1450x over previous
"""Trainium2 Bass kernel for nn_Net_SDE: 48-step neural SDE Monte-Carlo pricer.

Data-parallel over the 131072 MC samples across 8 NeuronCores (16384/core).

Per core the 48 SDE steps run as a For_i loop over NI=4 iterations of a
12-step unrolled body (measured sweet spot: a 73K-instruction fully unrolled
program runs markedly slower per step than a mid-size body looped, while very
small bodies pay a large per-back-edge cost).  Each step is processed
in four 4096-sample "quarters" so that the inter-step boundary work (state
update, layout shuffles) of quarter q hides under the MLP streams of
quarter q+1.

Within a quarter, the 4 MLPs are evaluated layer-wise as weight-stationary
streams of 8 N=512 fp16 matmuls into four [128, 1024] (2-bank) PSUM tiles;
each PSUM tile is drained by one FD=1024 bias+relu op, alternating between
the vector (DVE) and scalar (ACT) engines — the two drain engines together
are the throughput limit of the whole kernel.  The first layer is 2-way
row-tiled (tile_position=(0|32, 0), K=2) and the output layer uses M=4
zero-padded weights PSUM-accumulated across the 4 nets so all outputs land
on contiguous psum partitions 0-3.

Payoffs are computed after every maturity step into per-(iteration, step)
DRAM slots (uniform body; the host gathers slots per the runtime `indices`).
Only call partials relu(S-K) and sum(S) are computed on-device (21 ops per
step instead of 40); the host derives puts exactly via put-call parity:
relu(K-S) = relu(S-K) - (S-K).
"""
import numpy as np
from contextlib import ExitStack

import orjson

import concourse.bass as bass
import concourse.tile as tile
from concourse import mybir

F16 = mybir.dt.float16
F32 = mybir.dt.float32
AF = mybir.ActivationFunctionType
OP = mybir.AluOpType

MC = 131072
N_STEPS = 48
N_CORES = 8
MCL = MC // N_CORES          # 16384 samples per core
GF = 128                     # state grid: [128 partitions, GF free]
NQ = 4                       # quarters per step
QF = GF // NQ                # grid cols per quarter (32 -> 4096 samples)
QS = 128 * QF                # samples per quarter
CH = 512                     # matmul moving-dim chunk
PMW = 1024                   # psum tile free width (2 banks)
NSEG = QS // PMW             # psum tiles per (net, layer) stream (4)
CPS = PMW // CH              # matmul chunks per psum tile (2)
U_STEPS = 12                 # unrolled steps per For_i iteration

STRIKES_CALL = np.array([100., 105., 110., 115., 120., 125., 130., 135., 140., 145.], np.float32)
STRIKES_PUT = np.array([55., 60., 65., 70., 75., 80., 85., 90., 95., 100.], np.float32)


# ---------------------------------------------------------------------------
# Workaround: this walrus build accepts only ONE sync-wait command per
# instruction. Split any instruction with more waits into preceding
# same-engine Drain (ctrl no-op) instructions, one wait each — same-engine
# FIFO order makes this semantically identical.
def _split_sync_waits(bir_json: bytes) -> bytes:
    bir = orjson.loads(bir_json)
    for fn in bir.get("functions", []):
        for bb in fn.get("blocks", []):
            out = []
            changed = False
            for ins in bb.get("instructions", []):
                si = ins.get("sync_info") or {}
                waits = si.get("on_wait") or []
                if len(waits) > 1:
                    changed = True
                    for ci, w in enumerate(waits[:-1]):
                        out.append({
                            "name": f"{ins['name']}_sw{ci}",
                            "opcode": "Drain",
                            "engine": ins.get("engine", "SP"),
                            "ins": [], "outs": [],
                            "debug": ins.get("debug"),
                            "sync_info": {"on_update": [], "on_wait": [w]},
                        })
                    si["on_wait"] = waits[-1:]
                    ins["sync_info"] = si
                out.append(ins)
            if changed:
                bb["instructions"] = out
    return orjson.dumps(bir)


def _install_sync_split():
    import concourse.bass_utils as bu
    import concourse.bass2jax as b2j
    if getattr(bu, "_sync_split_installed", False):
        return
    orig = bu.compile_bir_kernel

    def patched(bir_json, tmpdir, neff_name="file.neff"):
        return orig(_split_sync_waits(bir_json), tmpdir, neff_name=neff_name)

    bu.compile_bir_kernel = patched
    bu._sync_split_installed = True
    if getattr(b2j, "compile_bir_kernel", None) is orig:
        b2j.compile_bir_kernel = patched


def build_nc(payoff_us, c0, bo0, bo1h, bo2, bo3, cS, cV, n_steps=N_STEPS,
             u_steps=None, repeat=1):
    """Build the single-core Bass program (SPMD: all cores run the same code).

    payoff_us: sorted in-body step offsets u that need payoff slots (union
    over iterations). c0 = 1 + r*h. bo* = output biases (net 1 pre-scaled
    by h).
    """
    if u_steps is None:
        u_steps = U_STEPS if n_steps % U_STEPS == 0 else n_steps
    assert n_steps % u_steps == 0
    NI = n_steps // u_steps
    U = u_steps

    nc = bass.Bass()

    z_in = nc.declare_dram_parameter("z", [NI, U * MCL], F32, isOutput=False)
    z1_in = nc.declare_dram_parameter("z1", [NI, U * MCL], F32, isOutput=False)
    wiT_in = nc.declare_dram_parameter("wiT", [128, 256], F16, isOutput=False)
    whT_in = nc.declare_dram_parameter("whT", [128, 1536], F16, isOutput=False)
    woT_in = nc.declare_dram_parameter("woT", [128, 16], F16, isOutput=False)
    b1_in = nc.declare_dram_parameter("b1", [NI, 128, 4 * U], F32, isOutput=False)
    bh_in = nc.declare_dram_parameter("bh", [128, 12], F32, isOutput=False)
    strk_in = nc.declare_dram_parameter("strk", [128, 20], F32, isOutput=False)
    payoff_us = sorted(set(payoff_us))
    n_slots = max(1, len(payoff_us))
    slot_of = {u: i for i, u in enumerate(payoff_us)}
    acc_out = nc.declare_dram_parameter("acc", [NI, 128, 21 * n_slots], F32,
                                        isOutput=True)

    with tile.TileContext(nc) as tc, ExitStack() as ctx:
        consts = ctx.enter_context(tc.tile_pool(name="consts", bufs=1))
        persist = ctx.enter_context(tc.tile_pool(name="persist", bufs=1))
        hpool = ctx.enter_context(tc.tile_pool(name="hpool", bufs=2))
        inppool = ctx.enter_context(tc.tile_pool(name="inppool", bufs=5))
        orowpool = ctx.enter_context(tc.tile_pool(name="orowpool", bufs=2))
        outspool = ctx.enter_context(tc.tile_pool(name="outspool", bufs=3))
        zpool = ctx.enter_context(tc.tile_pool(name="zpool", bufs=1))
        updpool = ctx.enter_context(tc.tile_pool(name="updpool", bufs=2))
        junkpool = ctx.enter_context(tc.tile_pool(name="junkpool", bufs=4))
        psmm = ctx.enter_context(tc.tile_pool(name="psmm", bufs=4, space="PSUM"))

        # ---- constants ----
        wiT = consts.tile([128, 256], F16)
        nc.sync.dma_start(out=wiT, in_=wiT_in[:, :])
        whT = consts.tile([128, 1536], F16)
        nc.sync.dma_start(out=whT, in_=whT_in[:, :])
        woT = consts.tile([128, 16], F16)
        nc.sync.dma_start(out=woT, in_=woT_in[:, :])
        bh = consts.tile([128, 12], F32)
        nc.sync.dma_start(out=bh, in_=bh_in[:, :])
        strk = consts.tile([128, 20], F32)
        nc.sync.dma_start(out=strk, in_=strk_in[:, :])

        # ---- persistent state ----
        S = persist.tile([128, GF], F32)
        V = persist.tile([128, GF], F32)
        S16 = persist.tile([128, GF], F16)
        V16 = persist.tile([128, GF], F16)
        # step-0 first-layer inputs are persistent: the body's last step
        # writes them for the next For_i iteration to read
        inp0_0 = persist.tile([128, QS], F16)
        inp0_1 = persist.tile([128, QS], F16)
        inp0_2 = persist.tile([128, QS], F16)
        inp0_3 = persist.tile([128, QS], F16)
        inp0 = [inp0_0, inp0_1, inp0_2, inp0_3]
        acc_all = persist.tile([128, 21 * n_slots], F32)

        def reset_state():
            nc.vector.memset(S[:, :], cS)
            nc.vector.memset(V[:, :], cV)
            nc.vector.memset(S16[:, :], 0.0)
            nc.vector.memset(V16[:, :], 0.0)
            for q in range(NQ):
                nc.vector.memset(inp0[q][:, :], 0.0)

        reset_state()

        ua = updpool.tile([128, QF], F32, tag="ua")
        ub = updpool.tile([128, QF], F32, tag="ub")
        uc = updpool.tile([128, QF], F32, tag="uc")
        ud = updpool.tile([128, QF], F32, tag="ud")

        # drain engine alternation: DVE is slightly slower per drain, so it
        # takes 15 of every 32 drains.
        state = {"k": 0}

        def drain_relu(dst, pm_ap, bias_ap):
            k = state["k"]
            state["k"] += 1
            if (k * 15) // 32 != ((k + 1) * 15) // 32:
                nc.vector.tensor_scalar(out=dst, in0=pm_ap, scalar1=bias_ap,
                                        scalar2=0.0, op0=OP.add, op1=OP.max)
            else:
                nc.scalar.activation(dst, pm_ap, AF.Relu, bias=bias_ap, scale=1.0)

        def drain_copy(dst, pm_ap):
            k = state["k"]
            state["k"] += 1
            if (k * 15) // 32 != ((k + 1) * 15) // 32:
                nc.vector.tensor_copy(dst, pm_ap)
            else:
                nc.scalar.copy(dst, pm_ap)

        def emit_payoff(u):
            # 20x relu(S-K) partials + sum(S) into this step's acc_all slot.
            # ACT only: DVE's accum_out does not produce free-dim sums on
            # this stack (verified empirically — it returns the last value).
            base = 21 * slot_of[u]
            for j in range(20):
                junk = junkpool.tile([128, GF], F32, tag="junk")
                nc.scalar.activation(junk, S, AF.Relu, bias=strk[:, j:j + 1],
                                     scale=1.0, accum_out=acc_all[:, base + j:base + j + 1])
            junk = junkpool.tile([128, GF], F32, tag="junk")
            nc.scalar.activation(junk, S, AF.Copy,
                                 accum_out=acc_all[:, base + 20:base + 21])

        def sde_body(iv):
            inp_tiles = list(inp0)
            # per-iteration parameter loads (register-indexed DMAs; only a
            # handful per body — many bounds-checked dynamic DMAs per loop
            # break walrus AP lowering)
            b1 = zpool.tile([128, 4 * U], F32, tag="b1")
            nc.sync.dma_start(out=b1[:, :], in_=b1_in[bass.ds(iv, 1), :, :])
            z_all = zpool.tile([128, U * GF], F32, tag="zall")
            nc.sync.dma_start(out=z_all[:, :], in_=z_in[bass.ds(iv, 1), :])
            z1_all = zpool.tile([128, U * GF], F32, tag="z1all")
            nc.sync.dma_start(out=z1_all[:, :], in_=z1_in[bass.ds(iv, 1), :])
            for u in range(U):

                for q in range(NQ):
                    qs = slice(QF * q, QF * q + QF)
                    inp_q = inp_tiles[q]
                    # -- first layer: 2-way row-tiled K=2 (strips 0 and 32);
                    # net pairs (0,1) then (2,3) run concurrently on the PE,
                    # their psum tiles draining on DVE and ACT in parallel --
                    hcur = [None] * 4
                    for na, nb in ((0, 1), (2, 3)):
                        ha = hpool.tile([128, QS], F16, tag=f"h{na}")
                        hb = hpool.tile([128, QS], F16, tag=f"h{nb}")
                        for seg in range(NSEG):
                            pma = psmm.tile([128, PMW], F32, tag="pm")
                            pmb = psmm.tile([128, PMW], F32, tag="pm")
                            for c in range(CPS):
                                col = seg * PMW + c * CH
                                for n, pm in ((na, pma), (nb, pmb)):
                                    strip = 32 * (n % 2)
                                    wcol = 128 * (n // 2)
                                    nc.tensor.matmul(
                                        pm[:, c * CH:(c + 1) * CH],
                                        lhsT=wiT[strip:strip + 2, wcol:wcol + 128],
                                        rhs=inp_q[strip:strip + 2, col:col + CH],
                                        start=True, stop=True,
                                        tile_position=(strip, 0))
                            for n, pm, hn in ((na, pma, ha), (nb, pmb, hb)):
                                drain_relu(hn[:, seg * PMW:(seg + 1) * PMW],
                                           pm[:, :], b1[:, 4 * u + n:4 * u + n + 1])
                        hcur[na] = ha
                        hcur[nb] = hb
                    # -- hidden layers --
                    for l in range(3):
                        for n in range(4):
                            hn = hpool.tile([128, QS], F16, tag=f"h{n}")
                            w_sl = slice((n * 3 + l) * 128, (n * 3 + l + 1) * 128)
                            for seg in range(NSEG):
                                pm = psmm.tile([128, PMW], F32, tag="pm")
                                for c in range(CPS):
                                    col = seg * PMW + c * CH
                                    nc.tensor.matmul(
                                        pm[:, c * CH:(c + 1) * CH],
                                        lhsT=whT[:, w_sl],
                                        rhs=hcur[n][:, col:col + CH],
                                        start=True, stop=True)
                                drain_relu(hn[:, seg * PMW:(seg + 1) * PMW],
                                           pm[:, :], bh[:, n * 3 + l:n * 3 + l + 1])
                            hcur[n] = hn
                    # -- output layer: M=4 zero-padded weights, accumulated
                    # across the 4 nets so outputs land on partitions 0-3 --
                    orow = orowpool.tile([4, QS], F32, tag="orow")
                    for seg in range(NSEG):
                        pmo = psmm.tile([128, PMW], F32, tag="pm")
                        for c in range(CPS):
                            for n in range(4):
                                col = seg * PMW + c * CH
                                nc.tensor.matmul(
                                    pmo[0:4, c * CH:(c + 1) * CH],
                                    lhsT=woT[:, 4 * n:4 * n + 4],
                                    rhs=hcur[n][:, col:col + CH],
                                    start=(n == 0), stop=(n == 3))
                        drain_copy(orow[0:4, seg * PMW:(seg + 1) * PMW],
                                   pmo[0:4, :])
                    # -- scatter net outputs back to the state grid --
                    og = []
                    for n in range(4):
                        g = outspool.tile([128, QF], F32, tag=f"og{n}")
                        nc.sync.dma_start(out=g[:, :], in_=orow[n:n + 1, :])
                        og.append(g)
                    # -- state update (fp32, DVE) --
                    Ssl = S[:, qs]
                    Vsl = V[:, qs]
                    zsl = z_all[:, u * GF + QF * q:u * GF + QF * q + QF]
                    z1sl = z1_all[:, u * GF + QF * q:u * GF + QF * q + QF]
                    # S' = relu(c0*S + (diff+bo0)*dW)
                    nc.vector.scalar_tensor_tensor(out=ua, in0=og[0], scalar=bo0,
                                                   in1=zsl, op0=OP.add, op1=OP.mult)
                    nc.vector.scalar_tensor_tensor(out=ub, in0=Ssl, scalar=c0,
                                                   in1=ua, op0=OP.mult, op1=OP.add)
                    nc.vector.tensor_scalar(out=Ssl, in0=ub, scalar1=0.0,
                                            scalar2=None, op0=OP.max)
                    # V' = V + (driftV*h+bo1h) + (diffV+bo2)*dW + (diffV1+bo3)*dW1
                    nc.vector.scalar_tensor_tensor(out=ua, in0=og[1], scalar=bo1h,
                                                   in1=Vsl, op0=OP.add, op1=OP.add)
                    nc.vector.scalar_tensor_tensor(out=ub, in0=og[2], scalar=bo2,
                                                   in1=zsl, op0=OP.add, op1=OP.mult)
                    nc.vector.scalar_tensor_tensor(out=uc, in0=og[3], scalar=bo3,
                                                   in1=z1sl, op0=OP.add, op1=OP.mult)
                    nc.vector.tensor_tensor(out=ud, in0=ua, in1=ub, op=OP.add)
                    nc.vector.tensor_tensor(out=Vsl, in0=ud, in1=uc, op=OP.add)
                    # centered fp16 copies for the next step's first layer
                    nc.vector.tensor_scalar(out=S16[:, qs], in0=Ssl, scalar1=cS,
                                            scalar2=None, op0=OP.subtract)
                    nc.vector.tensor_scalar(out=V16[:, qs], in0=Vsl, scalar1=cV,
                                            scalar2=None, op0=OP.subtract)
                    # next step's first-layer input rows (2 replicas for the
                    # 2-way row-tiled first layer); the last step writes the
                    # persistent tiles read by the next For_i iteration
                    if u + 1 < U:
                        ninp = inppool.tile([128, QS], F16, tag="inp")
                    else:
                        ninp = inp0[q]
                    for strip in (0, 32):
                        nc.sync.dma_start(out=ninp[strip:strip + 1, :],
                                          in_=S16[:, qs])
                        nc.sync.dma_start(out=ninp[strip + 1:strip + 2, :],
                                          in_=V16[:, qs])
                    inp_tiles[q] = ninp

                if u in payoff_us:
                    emit_payoff(u)
            nc.sync.dma_start(out=acc_out[bass.ds(iv, 1), :, :], in_=acc_all)

        rep_ctx = (tc.For_i(0, repeat, 1) if repeat > 1 else None)
        if rep_ctx is not None:
            rep_ctx.__enter__()
            reset_state()
        with tc.For_i(0, NI, 1) as iv:
            sde_body(iv)
        if rep_ctx is not None:
            rep_ctx.__exit__(None, None, None)

    return nc


def _prep_inputs(S0, V0, rate, z, z1, indices, timegrid, Wi, bi, Wh, bh, Wo, bo,
                 n_steps=None, u_steps=None):
    """Host-side preprocessing. Returns (build args, per-core inputs, disc,
    idx_steps)."""
    S0v = float(np.asarray(S0).reshape(-1)[0])
    V0v = float(np.asarray(V0).reshape(-1)[0])
    r = float(np.asarray(rate).reshape(-1)[0])
    z = np.asarray(z, np.float32)
    z1 = np.asarray(z1, np.float32)
    if n_steps is None:
        n_steps = z.shape[1]
    if u_steps is None:
        u_steps = U_STEPS if n_steps % U_STEPS == 0 else n_steps
    NI = n_steps // u_steps
    U = u_steps
    tg = np.asarray(timegrid, np.float64)
    h = float(tg[1] - tg[0])
    sqh = float(np.sqrt(h))
    c0 = 1.0 + r * h

    Wi = np.asarray(Wi, np.float32)
    bi = np.asarray(bi, np.float32)
    Wh = np.asarray(Wh, np.float32)
    bhv = np.asarray(bh, np.float32)
    Wo = np.asarray(Wo, np.float32).copy()
    bo = np.asarray(bo, np.float32).copy()
    # driftV net (index 1) is only ever used multiplied by h -> fold h into it
    Wo[1] *= h
    bo0, bo1h, bo2, bo3 = (float(bo[0, 0]), float(bo[1, 0]) * h,
                           float(bo[2, 0]), float(bo[3, 0]))

    cS, cV = S0v, V0v    # centering constants for fp16 inputs
    # first-layer bias with t-term and centering folded in: [4, n_steps, 128]
    t_vals = tg[:n_steps].astype(np.float32)
    b1 = (bi[:, None, :] + t_vals[None, :, None] * Wi[:, 0][:, None, :]
          + cS * Wi[:, 1][:, None, :] + cV * Wi[:, 2][:, None, :])
    # device layout: [NI, 128, 4U] with slab i, col u*4+n for step t = i*U+u
    b1_f_t_n = b1.transpose(2, 1, 0)                     # [128, n_steps, 4]
    b1_dev = np.ascontiguousarray(
        b1_f_t_n.reshape(128, NI, 4 * U).transpose(1, 0, 2), np.float32)

    # first-layer weights, 2-way row strips: net n lives at partition rows
    # 32*(n%2).. and weight cols 128*(n//2)..
    wiT_dev = np.zeros((128, 256), np.float16)
    for n in range(4):
        strip, wcol = 32 * (n % 2), 128 * (n // 2)
        wiT_dev[strip:strip + 2, wcol:wcol + 128] = Wi[n, 1:3, :].astype(np.float16)
    whT_dev = np.ascontiguousarray(
        Wh.transpose(2, 0, 1, 3).reshape(128, 12 * 128), np.float16)
    # output weights, zero-padded to M=4 per net: col 4n+j is Wo[n] when
    # j == n else 0 (outputs accumulate on psum partitions 0-3)
    woT_dev = np.zeros((128, 16), np.float16)
    for n in range(4):
        woT_dev[:, 4 * n + n] = Wo[n, :, 0].astype(np.float16)
    bh_dev = np.ascontiguousarray(bhv.transpose(2, 0, 1).reshape(128, 12), np.float32)

    strk_dev = np.ascontiguousarray(
        np.tile(np.concatenate([-STRIKES_CALL, -STRIKES_PUT])[None, :], (128, 1)),
        np.float32)

    idx = np.asarray(indices).astype(np.int64).reshape(-1)
    idx_steps = [int((v - 1) % n_steps) for v in idx]
    payoff_us = sorted({st % U for st in idx_steps})
    disc = np.exp(-r * 2.0 * idx.astype(np.float64) / n_steps)

    def z_layout(zc):
        # [MCL, n_steps] -> [NI, p, u, f] so that per-step grid slices of the
        # SBUF slab are static: slab[p, u*128 + f] = z[p*128+f, iv*U+u]
        a = zc.reshape(128, 128, NI, U)          # [p, f, iv, u]
        return np.ascontiguousarray(
            a.transpose(2, 0, 3, 1).reshape(NI, U * MCL), np.float32)

    in_maps = []
    for k in range(N_CORES):
        sl = slice(k * MCL, (k + 1) * MCL)
        zt = (z[sl, :n_steps] * sqh).astype(np.float32)
        z1t = (z1[sl, :n_steps] * sqh).astype(np.float32)
        in_maps.append({
            "z": z_layout(zt),
            "z1": z_layout(z1t),
            "wiT": wiT_dev, "whT": whT_dev, "woT": woT_dev,
            "b1": b1_dev, "bh": bh_dev, "strk": strk_dev,
        })
    build_args = dict(payoff_us=payoff_us, c0=c0, bo0=bo0, bo1h=bo1h,
                      bo2=bo2, bo3=bo3, cS=cS, cV=cV, n_steps=n_steps,
                      u_steps=U)
    return build_args, in_maps, disc, idx_steps


def _combine(results, disc, idx_steps, n_steps, u_steps):
    """Gather per-step slots into the [96, 10] output via put-call parity."""
    U = u_steps
    NI = n_steps // U
    payoff_us = sorted({st % U for st in idx_steps})
    slot_of = {u: i for i, u in enumerate(payoff_us)}
    n_slots = max(1, len(payoff_us))
    total = np.zeros((NI, 128, 21 * n_slots), np.float64)
    for res in results:
        total += np.asarray(res["acc"], np.float64)
    cols = total.sum(axis=1).reshape(NI, n_slots, 21)
    n_mat = len(idx_steps)
    calls_c = np.zeros((n_mat, 10))
    calls_p = np.zeros((n_mat, 10))
    sumS = np.zeros((n_mat, 1))
    for m, st in enumerate(idx_steps):
        row = cols[st // U, slot_of[st % U]]
        calls_c[m] = row[0:10]
        calls_p[m] = row[10:20]
        sumS[m, 0] = row[20]
    kc = STRIKES_CALL.astype(np.float64)[None, :]
    kp = STRIKES_PUT.astype(np.float64)[None, :]
    # relu(K - S) = relu(S - K) - S + K  (summed over MC samples)
    puts_c = calls_c - sumS + MC * kc
    puts_p = calls_p - sumS + MC * kp
    out = np.concatenate([calls_c, puts_p, calls_p, puts_c], axis=0) / MC
    out = out * np.concatenate([disc] * 4)[:, None]
    return out.astype(np.float32)


def kernel(**inputs) -> np.ndarray:
    from concourse.bass_utils import run_bass_kernel_spmd
    _install_sync_split()
    build_args, in_maps, disc, idx_steps = _prep_inputs(**inputs)
    nc = build_nc(**build_args)
    res = run_bass_kernel_spmd(nc, in_maps, list(range(N_CORES)))
    return _combine(res.results, disc, idx_steps, build_args["n_steps"],
                    build_args["u_steps"])
